# revision 9
# baseline (speedup 1.0000x reference)
"""V3 Trainium Bass kernel for nn_AdaptiveMoodCoherenceHysteresis.

Math (triad term == 1 identically):
  S[f] = sum_t sin(ph[f,t]);  C[f] = sum_t cos(ph[f,t])
  plv2[f] = S^2 + C^2
  coh = sum_f plv2^(1/4) / (F*sqrt(T))
  out = prev_coh + alpha*(coh - prev_coh), alpha via tanh hysteresis

V3 structure (single-shot latency focused):
  - range reduction per chunk on DVE: u = x/2pi (ts 2x), yt = u+MAGIC (ts 2x),
    nd = (yt-MAGIC)-u = -d (stt 1x), a = nd & 0x7fffffff = |d| (ts int 2x)
  - ACT: sin(2pi*nd) -> -S accum; sin(pi/2 - 2pi*a) -> C accum (sign of S
    is globally flipped; S enters only squared)
  - PE: ps64[64,2] = foldm^T @ acc[:, 2i:2i+2] accumulated over chunks
    (foldm stationary), then ACT Square+accum -> plv2[64,1]
  - DVE: 9-op quartic root -> s4[64,1]; PE ones-reduce -> pf[1,1] PSUM
  - ACT chain (Identity/Abs/Tanh, all in silu_and_others with sin) -> out
  - SP issues all input DMAs on one HWDGE ring (one sem, FIFO), out DMA at end
"""
import numpy as np
import concourse.bass as bass
import concourse.mybir as mybir
from concourse.bass_utils import run_bass_kernel_spmd

F, T = 64, 2048
P = 128
W = T // 2                   # free-dim elems per partition (two halves stacked)
N_CORES = 8

ALPHA_MIN = 0.08
ALPHA_MAX = 0.45
BETA = 0.12
SIG_SLOPE = 8.0
SIG_OFFSET = 1.5

TWO_PI = 2.0 * np.pi
INV_2PI = 1.0 / TWO_PI
MAGIC = 1.5 * 2.0 ** 23
K5 = 1332500000              # bit seed: q ~= x^(-1/4) = K5 - (bits >> 2)
QCORR = 0.701939             # sawtooth-mean correction for s4 = x*q^3
A = mybir.AluOpType
AF = mybir.ActivationFunctionType
F32 = mybir.dt.float32
I32 = mybir.dt.int32

# prev columns (host-precomputed per core)
NPC, T2, PC = range(3)


def build(widths=(352, 672), pool_mask=False, act_dma=False):
    ws = list(widths)
    assert sum(ws) == W
    k = len(ws)
    offs = np.cumsum([0] + ws).tolist()

    _om = bass.BassGpSimd.memset

    def _skip_unused(self, ap, value):
        if ap.tensor.dtype in (mybir.dt.bfloat16, mybir.dt.uint8):
            return None
        # route needed const memsets to DVE: Pool is the startup-barrier
        # straggler, DVE reaches the barrier earlier
        return self.bass.vector.memset(ap, value)

    bass.BassGpSimd.memset = _skip_unused
    try:
        nc = bass.Bass()
    finally:
        bass.BassGpSimd.memset = _om
    ph_in = nc.declare_dram_parameter("phases", [F, T], F32, isOutput=False)
    # aux[:, 0:64] = fold matrix (1.0 where (f - p) % 64 == 0); aux[0, 64:68]
    # = host-precomputed prev scalars. One DMA, no GPSIMD iota needed (first
    # GPSIMD use on HW can cost a ~6us ucode IRAM load)
    aux_in = nc.declare_dram_parameter("aux", [P, F + 4], F32, isOutput=False)
    out_d = nc.declare_dram_parameter("out", [1, 1], F32, isOutput=True)

    ph = ph_in[:].rearrange("f (h t) -> f h t", h=2).rearrange("f h t -> h f t")

    c2 = BETA * (ALPHA_MAX - ALPHA_MIN)
    kcoh = float(QCORR / (F * np.sqrt(T)))

    from contextlib import ExitStack
    with ExitStack() as stack:
        def sb(name, shape, dtype=F32):
            return stack.enter_context(nc.sbuf_tensor(name, shape, dtype))

        xb = [sb(f"x{j}", [P, ws[j]]) for j in range(k)]
        ub = sb("ub", [P, max(ws)])
        ytb = sb("ytb", [P, max(ws)])
        ndb = [sb(f"nd{j}", [P, ws[j]]) for j in range(k)]
        ab = [sb(f"a{j}", [P, ws[j]]) for j in range(k)]

        acc = sb("acc", [P, 2 * k])
        aux = sb("aux_sb", [P, F + 4])
        halfpi = sb("halfpi", [P, 1])
        btanh = sb("btanh", [1, 1])
        row = sb("row", [1, 128])
        sq = sb("sq", [1, 128])
        plv2 = sb("plv2", [1, F])
        nt1 = sb("nt1", [1, F])
        nt2 = sb("nt2", [1, F])
        nt3 = sb("nt3", [1, F])
        sc = sb("sc", [1, 8])

        # scr0 padded to exactly 2 PSUM banks (4KB/partition) so ps64/pf land
        # in a bank no activation streams over while PE accumulation is live
        scr0 = stack.enter_context(nc.psum_tensor("scr0", [P, 1024], F32))
        psS = stack.enter_context(nc.psum_tensor("psS", [1, F], F32))
        psC = stack.enter_context(nc.psum_tensor("psC", [1, F], F32))

        ch = [stack.enter_context(nc.semaphore(f"ch{j}")) for j in range(k)]
        scal = stack.enter_context(nc.semaphore("scal"))
        vd = stack.enter_context(nc.semaphore("vd"))      # DVE progress
        act_s = stack.enter_context(nc.semaphore("act_s"))
        pe_s = stack.enter_context(nc.semaphore("pe_s"))
        g = stack.enter_context(nc.semaphore("g"))
        block = stack.enter_context(nc.Block(no_gpsimd_drain=True))

        @block.sync
        def _(sync):
            for j in range(k):
                if act_dma and j % 2 == 1:
                    continue
                sync.dma_start(
                    xb[j][:], ph[:, :, offs[j]:offs[j] + ws[j]]
                ).then_inc(ch[j], 16)
            sync.dma_start(aux[:], aux_in[:]).then_inc(scal, 16)
            sync.wait_ge(act_s, 2 * k + 1)
            sync.dma_start(out_d[:], sc[0:1, 5:6]).then_inc(g, 16)

        @block.vector
        def _(vector):
            vector.memset(halfpi[:], float(np.pi / 2))
            vector.memset(btanh[:], float(-SIG_OFFSET / 2.0))
            for j in range(k):
                w = ws[j]
                vector.wait_ge(ch[j], 16)
                vector.tensor_scalar(ub[:, :w], xb[j][:], INV_2PI, None,
                                     A.mult)
                vector.tensor_scalar(ytb[:, :w], ub[:, :w], MAGIC, None,
                                     A.add)
                vector.scalar_tensor_tensor(
                    ndb[j][:], ytb[:, :w], MAGIC, ub[:, :w],
                    A.subtract, A.subtract,
                ).then_inc(vd, 1)
                vector.tensor_scalar(ab[j][:].bitcast(I32),
                                     ndb[j][:].bitcast(I32),
                                     0x7FFFFFFF, None,
                                     A.bitwise_and).then_inc(vd, 1)
            # tail (V2-proven [1,F] layout): fold rows -> plv2 -> x^(1/4)
            vector.wait_ge(pe_s, 1)
            vector.tensor_copy(row[0:1, 0:F], psS[:])
            vector.tensor_tensor(sq[0:1, 0:F], row[0:1, 0:F], row[0:1, 0:F],
                                 A.mult)
            vector.wait_ge(pe_s, 2)
            vector.tensor_copy(row[0:1, F:128], psC[:])
            vector.tensor_tensor(sq[0:1, F:128], row[0:1, F:128],
                                 row[0:1, F:128], A.mult)
            vector.tensor_tensor(plv2[:], sq[0:1, 0:F], sq[0:1, F:128],
                                 A.add)
            vector.tensor_scalar(nt1[:].bitcast(I32), plv2[:].bitcast(I32),
                                 2, None, A.arith_shift_right)
            vector.tensor_scalar(nt1[:].bitcast(I32), nt1[:].bitcast(I32),
                                 -1, K5, A.mult, A.add)
            vector.tensor_tensor(nt2[:], nt1[:], nt1[:], A.mult)
            vector.tensor_tensor(nt3[:], plv2[:], nt1[:], A.mult)
            vector.scalar_tensor_tensor(
                nt3[:], nt3[:], 1.0, nt2[:], A.mult, A.mult,
                accum_out=sc[0:1, 0:1],
            ).then_inc(vd, 1)

        @block.tensor
        def _(tensor):
            tensor.wait_ge(scal, 16)
            for j in range(k):
                tensor.wait_ge(act_s, 2 * j + 1)
                mmS = tensor.matmul(psS[:], acc[:, 2 * j:2 * j + 1],
                                    aux[:, 0:F],
                                    start=(j == 0), stop=(j == k - 1))
                if j == k - 1:
                    mmS.then_inc(pe_s, 1)
                tensor.wait_ge(act_s, 2 * j + 2)
                mm = tensor.matmul(psC[:], acc[:, 2 * j + 1:2 * j + 2],
                                   aux[:, 0:F],
                                   start=(j == 0), stop=(j == k - 1))
                if j == k - 1:
                    mm.then_inc(pe_s, 1)

        @block.scalar
        def _(scalar):
            if act_dma:
                # odd chunks ride ACT's own HWDGE ring, overlapping SP's
                # SEQ+descriptor-gen for chunk0; issued before the dummy Sin
                # so the implicit table load doesn't delay them
                for j in range(k):
                    if j % 2 == 1:
                        scalar.dma_start(
                            xb[j][:], ph[:, :, offs[j]:offs[j] + ws[j]]
                        ).then_inc(ch[j], 16)
            # dummy Sin: walrus places the table load here, overlapping DMAs
            zp = nc.const_aps.aps[(F32, 0.0)]
            scalar.activation(scr0[0:1, 0:1], zp[0:1, 0:1], AF.Sin,
                              bias=0.0, scale=1.0)
            for j in range(k):
                scalar.wait_ge(vd, 2 * j + 1)
                scalar.activation(scr0[:, :ws[j]], ndb[j][:], AF.Sin,
                                  bias=0.0, scale=TWO_PI,
                                  accum_out=acc[:, 2 * j:2 * j + 1]
                                  ).then_inc(act_s, 1)
                scalar.wait_ge(vd, 2 * j + 2)
                scalar.activation(scr0[:, :ws[j]], ab[j][:], AF.Sin,
                                  bias=halfpi[:], scale=-TWO_PI,
                                  accum_out=acc[:, 2 * j + 1:2 * j + 2]
                                  ).then_inc(act_s, 1)
            # hysteresis chain on sc[0] = sum_f plv2^(1/4)
            scalar.wait_ge(vd, 2 * k + 1)
            scalar.wait_ge(scal, 16)
            scalar.activation(sc[0:1, 1:2], sc[0:1, 0:1], AF.Identity,
                              bias=aux[0:1, F + NPC:F + NPC + 1], scale=kcoh)
            scalar.activation(sc[0:1, 2:3], sc[0:1, 1:2],
                              AF.Abs, bias=0.0, scale=1.0)
            scalar.activation(sc[0:1, 3:4], sc[0:1, 2:3],
                              AF.Tanh, bias=btanh[:],
                              scale=SIG_SLOPE / 2.0)
            scalar.activation(sc[0:1, 4:5], sc[0:1, 3:4],
                              AF.Identity, bias=aux[0:1, F + T2:F + T2 + 1],
                              scale=0.5 * c2)
            scalar.activation(sc[0:1, 5:6], sc[0:1, 1:2],
                              AF.Identity, bias=aux[0:1, F + PC:F + PC + 1],
                              scale=sc[0:1, 4:5]).then_inc(act_s, 1)

    return nc


_cache = {}


def _get_nc(widths=(352, 672), pool_mask=False, act_dma=False):
    key = (tuple(widths), pool_mask, act_dma)
    if key not in _cache:
        _cache[key] = build(widths=tuple(widths), pool_mask=pool_mask,
                            act_dma=act_dma)
    return _cache[key]


_FOLDM = None


def _foldm_host():
    global _FOLDM
    if _FOLDM is None:
        m = np.zeros((P, F), dtype=np.float32)
        for p in range(P):
            m[p, p % F] = 1.0
        _FOLDM = m
    return _FOLDM


def _aux_input(pc_b, pa_b):
    c2 = BETA * (ALPHA_MAX - ALPHA_MIN)
    t2 = (1.0 - BETA) * pa_b + BETA * ALPHA_MIN + 0.5 * c2
    aux = np.zeros((P, F + 4), dtype=np.float32)
    aux[:, 0:F] = _foldm_host()
    aux[0, F:F + 4] = [-pc_b, t2, pc_b, 0.0]
    return aux


def kernel(phases, prev_coh, prev_alpha):
    phases = np.ascontiguousarray(np.asarray(phases, dtype=np.float32))
    prev_coh = np.asarray(prev_coh, dtype=np.float32)
    prev_alpha = np.asarray(prev_alpha, dtype=np.float32)
    B = phases.shape[0]
    assert B == N_CORES and phases.shape[1:] == (F, T)

    nc = _get_nc()
    in_maps = [
        {"phases": phases[b], "aux": _aux_input(prev_coh[b], prev_alpha[b])}
        for b in range(B)
    ]
    res = run_bass_kernel_spmd(nc, in_maps, core_ids=list(range(N_CORES))).results
    return np.array([res[b]["out"][0, 0] for b in range(B)], dtype=np.float32)


# revision 10
# speedup vs baseline: 1.0400x; 1.0400x over previous
"""V3 Trainium Bass kernel for nn_AdaptiveMoodCoherenceHysteresis.

Math (triad term == 1 identically):
  S[f] = sum_t sin(ph[f,t]);  C[f] = sum_t cos(ph[f,t])
  plv2[f] = S^2 + C^2
  coh = sum_f plv2^(1/4) / (F*sqrt(T))
  out = prev_coh + alpha*(coh - prev_coh), alpha via tanh hysteresis

V3 structure (single-shot latency focused):
  - range reduction per chunk on DVE: u = x/2pi (ts 2x), yt = u+MAGIC (ts 2x),
    nd = (yt-MAGIC)-u = -d (stt 1x), a = nd & 0x7fffffff = |d| (ts int 2x)
  - ACT: sin(2pi*nd) -> -S accum; sin(pi/2 - 2pi*a) -> C accum (sign of S
    is globally flipped; S enters only squared)
  - PE: ps64[64,2] = foldm^T @ acc[:, 2i:2i+2] accumulated over chunks
    (foldm stationary), then ACT Square+accum -> plv2[64,1]
  - DVE: 9-op quartic root -> s4[64,1]; PE ones-reduce -> pf[1,1] PSUM
  - ACT chain (Identity/Abs/Tanh, all in silu_and_others with sin) -> out
  - SP issues all input DMAs on one HWDGE ring (one sem, FIFO), out DMA at end
"""
import numpy as np
import concourse.bass as bass
import concourse.mybir as mybir
from concourse.bass_utils import run_bass_kernel_spmd

F, T = 64, 2048
P = 128
W = T // 2                   # free-dim elems per partition (two halves stacked)
N_CORES = 8

ALPHA_MIN = 0.08
ALPHA_MAX = 0.45
BETA = 0.12
SIG_SLOPE = 8.0
SIG_OFFSET = 1.5

TWO_PI = 2.0 * np.pi
INV_2PI = 1.0 / TWO_PI
MAGIC = 1.5 * 2.0 ** 23
K5 = 1332500000              # bit seed: q ~= x^(-1/4) = K5 - (bits >> 2)
QCORR = 0.701939             # sawtooth-mean correction for s4 = x*q^3
A = mybir.AluOpType
AF = mybir.ActivationFunctionType
F32 = mybir.dt.float32
I32 = mybir.dt.int32

# prev columns (host-precomputed per core)
NPC, T2, PC = range(3)


def build(widths=(352, 672), pool_mask=False, act_dma=False):
    ws = list(widths)
    assert sum(ws) == W
    k = len(ws)
    offs = np.cumsum([0] + ws).tolist()

    _om = bass.BassGpSimd.memset
    _ob = bass.Bass.all_engine_barrier

    def _skip_unused(self, ap, value):
        if ap.tensor.dtype in (mybir.dt.bfloat16, mybir.dt.uint8):
            return None
        # route needed const memsets to DVE (their values only feed the
        # dummy-sin input, where any bits are fine)
        return self.bass.vector.memset(ap, value)

    # Skip the construction-time all-engine barrier: it only orders the
    # const-AP memsets against value-consumers, and this kernel has none;
    # all real cross-engine ordering is via explicit semaphores, which NRT
    # zeroes at load. Validated over 14+ fresh-process first-runs.
    bass.BassGpSimd.memset = _skip_unused
    bass.Bass.all_engine_barrier = lambda self, **kw: None
    try:
        nc = bass.Bass()
    finally:
        bass.BassGpSimd.memset = _om
        bass.Bass.all_engine_barrier = _ob
    ph_in = nc.declare_dram_parameter("phases", [F, T], F32, isOutput=False)
    # aux[:, 0:64] = fold matrix (1.0 where (f - p) % 64 == 0); aux[0, 64:68]
    # = host-precomputed prev scalars. One DMA, no GPSIMD iota needed (first
    # GPSIMD use on HW can cost a ~6us ucode IRAM load)
    aux_in = nc.declare_dram_parameter("aux", [P, F + 4], F32, isOutput=False)
    out_d = nc.declare_dram_parameter("out", [1, 1], F32, isOutput=True)

    ph = ph_in[:].rearrange("f (h t) -> f h t", h=2).rearrange("f h t -> h f t")

    c2 = BETA * (ALPHA_MAX - ALPHA_MIN)
    kcoh = float(QCORR / (F * np.sqrt(T)))

    from contextlib import ExitStack
    with ExitStack() as stack:
        def sb(name, shape, dtype=F32):
            return stack.enter_context(nc.sbuf_tensor(name, shape, dtype))

        xb = [sb(f"x{j}", [P, ws[j]]) for j in range(k)]
        ub = sb("ub", [P, max(ws)])
        ytb = sb("ytb", [P, max(ws)])
        ndb = [sb(f"nd{j}", [P, ws[j]]) for j in range(k)]
        ab = [sb(f"a{j}", [P, ws[j]]) for j in range(k)]

        acc = sb("acc", [P, 2 * k])
        aux = sb("aux_sb", [P, F + 4])
        halfpi = sb("halfpi", [P, 1])
        btanh = sb("btanh", [1, 1])
        row = sb("row", [1, 128])
        sq = sb("sq", [1, 128])
        plv2 = sb("plv2", [1, F])
        nt1 = sb("nt1", [1, F])
        nt2 = sb("nt2", [1, F])
        nt3 = sb("nt3", [1, F])
        sc = sb("sc", [1, 8])

        # scr0 padded to exactly 2 PSUM banks (4KB/partition) so ps64/pf land
        # in a bank no activation streams over while PE accumulation is live
        scr0 = stack.enter_context(nc.psum_tensor("scr0", [P, 1024], F32))
        psS = stack.enter_context(nc.psum_tensor("psS", [1, F], F32))
        psC = stack.enter_context(nc.psum_tensor("psC", [1, F], F32))

        ch = [stack.enter_context(nc.semaphore(f"ch{j}")) for j in range(k)]
        scal = stack.enter_context(nc.semaphore("scal"))
        vd = stack.enter_context(nc.semaphore("vd"))      # DVE progress
        act_s = stack.enter_context(nc.semaphore("act_s"))
        pe_s = stack.enter_context(nc.semaphore("pe_s"))
        g = stack.enter_context(nc.semaphore("g"))
        block = stack.enter_context(nc.Block(no_gpsimd_drain=True))

        @block.sync
        def _(sync):
            for j in range(k):
                if act_dma and j % 2 == 1:
                    continue
                sync.dma_start(
                    xb[j][:], ph[:, :, offs[j]:offs[j] + ws[j]]
                ).then_inc(ch[j], 16)
            sync.dma_start(aux[:], aux_in[:]).then_inc(scal, 16)
            sync.wait_ge(act_s, 2 * k + 1)
            sync.dma_start(out_d[:], sc[0:1, 5:6]).then_inc(g, 16)

        @block.vector
        def _(vector):
            vector.memset(halfpi[:], float(np.pi / 2))
            vector.memset(btanh[:], float(-SIG_OFFSET / 2.0))
            for j in range(k):
                w = ws[j]
                vector.wait_ge(ch[j], 16)
                vector.tensor_scalar(ub[:, :w], xb[j][:], INV_2PI, None,
                                     A.mult)
                vector.tensor_scalar(ytb[:, :w], ub[:, :w], MAGIC, None,
                                     A.add)
                vector.scalar_tensor_tensor(
                    ndb[j][:], ytb[:, :w], MAGIC, ub[:, :w],
                    A.subtract, A.subtract,
                ).then_inc(vd, 1)
                vector.tensor_scalar(ab[j][:].bitcast(I32),
                                     ndb[j][:].bitcast(I32),
                                     0x7FFFFFFF, None,
                                     A.bitwise_and).then_inc(vd, 1)
            # tail (V2-proven [1,F] layout): fold rows -> plv2 -> x^(1/4)
            vector.wait_ge(pe_s, 1)
            vector.tensor_copy(row[0:1, 0:F], psS[:])
            vector.tensor_tensor(sq[0:1, 0:F], row[0:1, 0:F], row[0:1, 0:F],
                                 A.mult)
            vector.wait_ge(pe_s, 2)
            vector.tensor_copy(row[0:1, F:128], psC[:])
            vector.tensor_tensor(sq[0:1, F:128], row[0:1, F:128],
                                 row[0:1, F:128], A.mult)
            vector.tensor_tensor(plv2[:], sq[0:1, 0:F], sq[0:1, F:128],
                                 A.add)
            vector.tensor_scalar(nt1[:].bitcast(I32), plv2[:].bitcast(I32),
                                 2, None, A.arith_shift_right)
            vector.tensor_scalar(nt1[:].bitcast(I32), nt1[:].bitcast(I32),
                                 -1, K5, A.mult, A.add)
            vector.tensor_tensor(nt2[:], nt1[:], nt1[:], A.mult)
            vector.tensor_tensor(nt3[:], plv2[:], nt1[:], A.mult)
            vector.scalar_tensor_tensor(
                nt3[:], nt3[:], 1.0, nt2[:], A.mult, A.mult,
                accum_out=sc[0:1, 0:1],
            ).then_inc(vd, 1)

        @block.tensor
        def _(tensor):
            tensor.wait_ge(scal, 16)
            for j in range(k):
                tensor.wait_ge(act_s, 2 * j + 1)
                mmS = tensor.matmul(psS[:], acc[:, 2 * j:2 * j + 1],
                                    aux[:, 0:F],
                                    start=(j == 0), stop=(j == k - 1))
                if j == k - 1:
                    mmS.then_inc(pe_s, 1)
                tensor.wait_ge(act_s, 2 * j + 2)
                mm = tensor.matmul(psC[:], acc[:, 2 * j + 1:2 * j + 2],
                                   aux[:, 0:F],
                                   start=(j == 0), stop=(j == k - 1))
                if j == k - 1:
                    mm.then_inc(pe_s, 1)

        @block.scalar
        def _(scalar):
            if act_dma:
                # odd chunks ride ACT's own HWDGE ring, overlapping SP's
                # SEQ+descriptor-gen for chunk0; issued before the dummy Sin
                # so the implicit table load doesn't delay them
                for j in range(k):
                    if j % 2 == 1:
                        scalar.dma_start(
                            xb[j][:], ph[:, :, offs[j]:offs[j] + ws[j]]
                        ).then_inc(ch[j], 16)
            # dummy Sin: walrus places the table load here, overlapping DMAs
            zp = nc.const_aps.aps[(F32, 0.0)]
            scalar.activation(scr0[0:1, 0:1], zp[0:1, 0:1], AF.Sin,
                              bias=0.0, scale=1.0)
            for j in range(k):
                scalar.wait_ge(vd, 2 * j + 1)
                scalar.activation(scr0[:, :ws[j]], ndb[j][:], AF.Sin,
                                  bias=0.0, scale=TWO_PI,
                                  accum_out=acc[:, 2 * j:2 * j + 1]
                                  ).then_inc(act_s, 1)
                scalar.wait_ge(vd, 2 * j + 2)
                scalar.activation(scr0[:, :ws[j]], ab[j][:], AF.Sin,
                                  bias=halfpi[:], scale=-TWO_PI,
                                  accum_out=acc[:, 2 * j + 1:2 * j + 2]
                                  ).then_inc(act_s, 1)
            # hysteresis chain on sc[0] = sum_f plv2^(1/4)
            scalar.wait_ge(vd, 2 * k + 1)
            scalar.wait_ge(scal, 16)
            scalar.activation(sc[0:1, 1:2], sc[0:1, 0:1], AF.Identity,
                              bias=aux[0:1, F + NPC:F + NPC + 1], scale=kcoh)
            scalar.activation(sc[0:1, 2:3], sc[0:1, 1:2],
                              AF.Abs, bias=0.0, scale=1.0)
            scalar.activation(sc[0:1, 3:4], sc[0:1, 2:3],
                              AF.Tanh, bias=btanh[:],
                              scale=SIG_SLOPE / 2.0)
            scalar.activation(sc[0:1, 4:5], sc[0:1, 3:4],
                              AF.Identity, bias=aux[0:1, F + T2:F + T2 + 1],
                              scale=0.5 * c2)
            scalar.activation(sc[0:1, 5:6], sc[0:1, 1:2],
                              AF.Identity, bias=aux[0:1, F + PC:F + PC + 1],
                              scale=sc[0:1, 4:5]).then_inc(act_s, 1)

    return nc


_cache = {}


def _get_nc(widths=(352, 672), pool_mask=False, act_dma=False):
    key = (tuple(widths), pool_mask, act_dma)
    if key not in _cache:
        _cache[key] = build(widths=tuple(widths), pool_mask=pool_mask,
                            act_dma=act_dma)
    return _cache[key]


_FOLDM = None


def _foldm_host():
    global _FOLDM
    if _FOLDM is None:
        m = np.zeros((P, F), dtype=np.float32)
        for p in range(P):
            m[p, p % F] = 1.0
        _FOLDM = m
    return _FOLDM


def _aux_input(pc_b, pa_b):
    c2 = BETA * (ALPHA_MAX - ALPHA_MIN)
    t2 = (1.0 - BETA) * pa_b + BETA * ALPHA_MIN + 0.5 * c2
    aux = np.zeros((P, F + 4), dtype=np.float32)
    aux[:, 0:F] = _foldm_host()
    aux[0, F:F + 4] = [-pc_b, t2, pc_b, 0.0]
    return aux


def kernel(phases, prev_coh, prev_alpha):
    phases = np.ascontiguousarray(np.asarray(phases, dtype=np.float32))
    prev_coh = np.asarray(prev_coh, dtype=np.float32)
    prev_alpha = np.asarray(prev_alpha, dtype=np.float32)
    B = phases.shape[0]
    assert B == N_CORES and phases.shape[1:] == (F, T)

    nc = _get_nc()
    in_maps = [
        {"phases": phases[b], "aux": _aux_input(prev_coh[b], prev_alpha[b])}
        for b in range(B)
    ]
    res = run_bass_kernel_spmd(nc, in_maps, core_ids=list(range(N_CORES))).results
    return np.array([res[b]["out"][0, 0] for b in range(B)], dtype=np.float32)
